# revision 1
# baseline (speedup 1.0000x reference)
"""Multi-head attention block (B=2, N=2048, C=1024, H=16, D=64) on 8
Trainium2 NeuronCores — fused-pipeline version.

Sharding: core c -> batch b = c//4, head-group g = c%4 (tensor-parallel over
heads within a batch, 4 heads per core). QKV weights column-sharded, w_proj
row-sharded; each core emits a partial [N, C] projection which the host sums
per batch and then adds b_proj.

Final version (v4a):
- bf16 pipeline end to end (inputs pre-cast on host), tol 2e-2 passes at ~3e-3.
- Single fused emission: K(all) -> V(all) -> Q(0), then per-q-slab attention
  with Q(s+1)/proj(s-1) units interleaved between head-pairs so ACT exp and
  DVE work overlap the QKV/proj matmuls.
- Head-pair QK^T row-packing: the two heads of a pair sit at partitions
  0-63 / 64-127, so their K=64-contraction QK^T matmuls carry disjoint PE
  row-groups (tile_position auto-derived from base_partition) and execute
  concurrently on the 128x128 array; one exp instruction covers both heads'
  S^T chunk ([128, 1024]); PV lags one chunk.
- x shipped host-swizzled as [128, slab, kc, 512] so each per-slab DMA is a
  contiguous 8KB run per partition.
- PSUM: 2 S^T rings (2 banks each) + 2 po + 2 shared qkv/proj banks.
"""

import sys

sys.path.insert(0, "/opt/trn_rl_repo")

import numpy as np

from contextlib import ExitStack

import concourse.bacc as bacc
import concourse.tile as tile
from concourse import mybir

F32 = mybir.dt.float32
BF16 = mybir.dt.bfloat16

N = 2048
C = 1024
HL = 4  # heads per core
D = 64
KC = C // 128  # 8 contraction chunks
NS = N // 512  # 4 n-supers / slabs
MC = N // 128  # 16 m-chunks


def build_attention_nc(mm_dtype=BF16, loop_iters=None, stag=False, nring=2, glen=2, probes=frozenset()):
    nc = bacc.Bacc(None, target_bir_lowering=False, debug=False)

    MMDT = mm_dtype
    xt = nc.dram_tensor("xt", [128, NS, KC, 512], MMDT, kind="ExternalInput")
    wq = nc.dram_tensor("wq", [C, 2, 128], MMDT, kind="ExternalInput")
    wk = nc.dram_tensor("wk", [C, 2, 128], MMDT, kind="ExternalInput")
    wv = nc.dram_tensor("wv", [C, 256], MMDT, kind="ExternalInput")
    bq = nc.dram_tensor("bq", [2, 128], F32, kind="ExternalInput")
    bk = nc.dram_tensor("bk", [2, 128], F32, kind="ExternalInput")
    bv = nc.dram_tensor("bv", [256], F32, kind="ExternalInput")
    wp = nc.dram_tensor("wp", [256, C], MMDT, kind="ExternalInput")
    onesv = nc.dram_tensor("onesv", [128, 64], MMDT, kind="ExternalInput")
    out = nc.dram_tensor("out", [N, C], MMDT, kind="ExternalOutput")

    with (
        tile.TileContext(nc) as tc,
        ExitStack() as ctx,
        nc.allow_low_precision(reason="bf16 matmul pipeline, tol 2e-2"),
    ):
        const = ctx.enter_context(tc.tile_pool(name="const", bufs=1))
        persist = ctx.enter_context(tc.tile_pool(name="persist", bufs=1))
        xt_pool = ctx.enter_context(tc.tile_pool(name="xt_pool", bufs=1))
        mm_ps = ctx.enter_context(tc.tile_pool(name="mm_ps", bufs=2, space="PSUM"))
        st_ps = ctx.enter_context(tc.tile_pool(name="st_ps", bufs=1, space="PSUM"))
        o_ps = ctx.enter_context(tc.tile_pool(name="o_ps", bufs=1, space="PSUM"))
        p_pool = ctx.enter_context(tc.tile_pool(name="p_pool", bufs=4))
        r_pool = ctx.enter_context(tc.tile_pool(name="r_pool", bufs=4))
        out_pool = ctx.enter_context(tc.tile_pool(name="out_pool", bufs=3))

        # --- constants / weights (emission order = DMA priority) ---
        wk_sb = const.tile([128, KC, 256], MMDT)
        wv_sb = const.tile([128, KC, 256], MMDT)
        wq_sb = const.tile([128, KC, 256], MMDT)
        nc.sync.dma_start(out=wk_sb, in_=wk.rearrange("(kc p) j m -> p kc (j m)", p=128))
        bk_sb = const.tile([128, 2], F32)
        nc.sync.dma_start(out=bk_sb, in_=bk.rearrange("j p -> p j"))
        nc.sync.dma_start(out=wv_sb, in_=wv.rearrange("(kc p) m -> p kc m", p=128))
        bv_rep = const.tile([128, 256], F32)
        nc.sync.dma_start(out=bv_rep, in_=bv[:].unsqueeze(0).partition_broadcast(128))
        nc.sync.dma_start(out=wq_sb, in_=wq.rearrange("(kc p) j m -> p kc (j m)", p=128))
        bq_sb = const.tile([128, 2], F32)
        nc.sync.dma_start(out=bq_sb, in_=bq.rearrange("j p -> p j"))
        wp_sb = const.tile([128, 2, C], MMDT)
        nc.sync.dma_start(out=wp_sb, in_=wp.rearrange("(kc p) n -> p kc n", p=128))

        # --- persistent intermediates ---
        qt_sb = persist.tile([128, 2, N], MMDT)  # [d(2 heads), pair, n]
        kt_sb = persist.tile([128, 2, N], MMDT)
        v_sb = persist.tile([128, MC, HL, 65], MMDT)  # [m, mc, head, d|1]
        ot_sb = persist.tile([128, 2, N], MMDT)  # [d(2 heads), hd-chunk, n]
        nc.sync.dma_start(
            out=v_sb[:, :, :, 64:65],
            in_=onesv.rearrange("p (a b c) -> p a b c", a=MC, b=HL, c=1),
        )

        def body():
            # x resident for all slabs, loaded per-slab for fine-grain deps
            xts = xt_pool.tile([128, NS, KC, 512], MMDT, tag="xts")
            if "xdma" not in probes:
                for s in range(NS):
                    nc.sync.dma_start(
                        out=xts[:, s, :, :],
                        in_=xt[:, s, :, :],
                    )

            def emit_qk(s, j, wsb, bsb, dst, nm):
                if "qkv" in probes:
                    return
                ps = mm_ps.tile([128, 512], F32, tag="mmps", name=f"{nm}{s}{j}")
                for kc in range(KC):
                    nc.tensor.matmul(
                        ps,
                        lhsT=wsb[:, kc, j * 128 : (j + 1) * 128],
                        rhs=xts[:, s, kc, :],
                        start=kc == 0,
                        stop=kc == KC - 1,
                    )
                nc.vector.tensor_scalar_add(
                    out=dst[:, j, s * 512 : (s + 1) * 512],
                    in0=ps,
                    scalar1=bsb[:, j : j + 1],
                )

            def emit_v(s, jj):
                if "qkv" in probes:
                    return
                ps = mm_ps.tile([128, 256], F32, tag="mmps", name=f"v{s}{jj}")
                for kc in range(KC):
                    nc.tensor.matmul(
                        ps,
                        lhsT=xts[:, s, kc, jj * 128 : (jj + 1) * 128],
                        rhs=wv_sb[:, kc, :],
                        start=kc == 0,
                        stop=kc == KC - 1,
                    )
                mc = s * 4 + jj
                nc.vector.tensor_add(
                    out=v_sb[:, mc, :, 0:64],
                    in0=ps.rearrange("p (h d) -> p h d", h=HL),
                    in1=bv_rep.rearrange("p (h d) -> p h d", h=HL),
                )

            def emit_proj(s, ntl):
                if "proj" in probes:
                    return
                nt = s * 4 + ntl
                for cc in range(2):
                    ps = mm_ps.tile([128, 512], F32, tag="mmps", name=f"pj{nt}{cc}")
                    for hdc in range(2):
                        nc.tensor.matmul(
                            ps,
                            lhsT=ot_sb[:, hdc, nt * 128 : (nt + 1) * 128],
                            rhs=wp_sb[:, hdc, cc * 512 : (cc + 1) * 512],
                            start=hdc == 0,
                            stop=hdc == 1,
                        )
                    so = out_pool.tile([128, 512], MMDT, tag="so")
                    nc.vector.tensor_copy(out=so, in_=ps)
                    nc.sync.dma_start(
                        out=out[nt * 128 : (nt + 1) * 128, cc * 512 : (cc + 1) * 512],
                        in_=so,
                    )

            # ---- prologue: K and V for all slabs (attention reads them for
            # every m-chunk, and Tile only records RAW deps on writes that
            # were traced earlier), then Q(0) ----
            for s in range(NS):
                for j in range(2):
                    emit_qk(s, j, wk_sb, bk_sb, kt_sb, "k")
            for s in range(NS):
                for jj in range(4):
                    emit_v(s, jj)
            for j in range(2):
                emit_qk(0, j, wq_sb, bq_sb, qt_sb, "q")

            # per-slab extra work units, interleaved between attention heads
            units = {
                0: [lambda j=j: emit_qk(1, j, wq_sb, bq_sb, qt_sb, "q") for j in range(2)],
                1: [lambda n=n: emit_proj(0, n) for n in range(4)]
                + [lambda j=j: emit_qk(2, j, wq_sb, bq_sb, qt_sb, "q") for j in range(2)],
                2: [lambda n=n: emit_proj(1, n) for n in range(4)]
                + [lambda j=j: emit_qk(3, j, wq_sb, bq_sb, qt_sb, "q") for j in range(2)],
                3: [lambda n=n: emit_proj(2, n) for n in range(4)],
            }

            # rings hold one m-chunk for BOTH heads of a pair: [128, hh, 512]
            rings = [
                st_ps.tile([128, 2, 512], F32, tag=f"ring{u}", name=f"ring{u}")
                for u in range(nring)
            ]

            for s in range(NS):
                ulist = units[s]
                # distribute units across the 2 head-pairs
                per_pair = [ulist[i::2] for i in range(2)]
                for j in range(2):
                    po2 = [
                        o_ps.tile([128, 512], F32, tag=f"po{hh}", name=f"po_s{s}j{j}h{hh}")
                        for hh in range(2)
                    ]

                    def emit_pv(prev):
                        if "pv" in probes:
                            return
                        pt, mc = prev
                        for hh in range(2):
                            nc.tensor.matmul(
                                po2[hh][0:65, :],
                                lhsT=v_sb[:, mc, 2 * j + hh, :],
                                rhs=pt[:, hh, :],
                                start=mc == 0,
                                stop=mc == MC - 1,
                            )

                    prev = None
                    for mc in range(MC):
                        ring = rings[mc % nring]
                        # both heads' QK^T back-to-back: base partitions 0/64
                        # -> disjoint PE row groups, run concurrently
                        for hh in range(2 if "qkt" not in probes else 0):
                            base = hh * 64
                            nc.tensor.matmul(
                                ring[:, hh, :],
                                lhsT=kt_sb[
                                    base : base + 64, j, mc * 128 : (mc + 1) * 128
                                ],
                                rhs=qt_sb[
                                    base : base + 64, j, s * 512 : (s + 1) * 512
                                ],
                                start=True,
                                stop=True,
                            )
                        pt = p_pool.tile([128, 2, 512], MMDT, tag="pt")
                        if "exp" not in probes:
                            nc.scalar.activation(
                                out=pt,
                                in_=ring,
                                func=mybir.ActivationFunctionType.Exp,
                            )
                        if prev is not None:
                            emit_pv(prev)
                        prev = (pt, mc)
                    emit_pv(prev)

                    for hh in range(2):
                        recip = r_pool.tile([1, 512], F32, tag="recip")
                        nc.vector.reciprocal(out=recip, in_=po2[hh][64:65, :])
                        rden_sb = r_pool.tile(
                            [64, 512], F32, tag="rden_sb", name=f"rd_s{s}j{j}h{hh}"
                        )
                        nc.gpsimd.partition_broadcast(rden_sb, recip)
                        nc.vector.tensor_mul(
                            out=ot_sb[
                                hh * 64 : hh * 64 + 64,
                                j,
                                s * 512 : (s + 1) * 512,
                            ],
                            in0=po2[hh][0:64, :],
                            in1=rden_sb,
                        )
                    for u in per_pair[j]:
                        u()

            # epilogue: proj of last slab
            for n in range(4):
                emit_proj(3, n)

        if loop_iters is None:
            body()
        else:
            with tc.For_i(0, loop_iters, 1, staggered_reset=stag):
                body()

    nc.compile()
    return nc


EMBED_DIM = 1024
NUM_HEADS = 16
HEAD_DIM = 64
HPC = 4

_CACHE = {}


def _bf16(a):
    import ml_dtypes

    return np.asarray(a, dtype=ml_dtypes.bfloat16)


def _make_in_maps(x, w_qkv, b_qkv, w_proj):
    scale = HEAD_DIM ** -0.5
    def _swizzle(xb):
        xt = np.ascontiguousarray(xb.T)  # [C, N]
        return _bf16(
            np.ascontiguousarray(
                xt.reshape(KC, 128, NS, 512).transpose(1, 2, 0, 3)
            )
        )

    xts = [_swizzle(x[b]) for b in range(2)]
    ones = _bf16(np.ones((128, 64), np.float32))
    in_maps = []
    for core in range(8):
        b, g = core // 4, core % 4
        cols = slice(g * HPC * HEAD_DIM, (g + 1) * HPC * HEAD_DIM)
        wq = (w_qkv[:, 0:C][:, cols] * scale).astype(np.float32)
        wk = w_qkv[:, C : 2 * C][:, cols].astype(np.float32)
        wv = w_qkv[:, 2 * C : 3 * C][:, cols].astype(np.float32)
        bq = (b_qkv[0:C][cols] * scale).astype(np.float32)
        bk = b_qkv[C : 2 * C][cols].astype(np.float32)
        bvv = b_qkv[2 * C : 3 * C][cols].astype(np.float32)
        wpm = np.ascontiguousarray(w_proj[cols.start : cols.stop, :]).astype(np.float32)
        in_maps.append(
            {
                "xt": xts[b],
                "wq": _bf16(np.ascontiguousarray(wq.reshape(C, 2, 128))),
                "wk": _bf16(np.ascontiguousarray(wk.reshape(C, 2, 128))),
                "wv": _bf16(np.ascontiguousarray(wv)),
                "bq": np.ascontiguousarray(bq.reshape(2, 128)),
                "bk": np.ascontiguousarray(bk.reshape(2, 128)),
                "bv": np.ascontiguousarray(bvv),
                "wp": _bf16(wpm),
                "onesv": ones,
            }
        )
    return in_maps


def kernel(x, w_qkv, b_qkv, w_proj, b_proj):
    from concourse.bass_utils import run_bass_kernel_spmd

    x = np.asarray(x)
    w_qkv = np.asarray(w_qkv)
    b_qkv = np.asarray(b_qkv)
    w_proj = np.asarray(w_proj)
    b_proj = np.asarray(b_proj)

    if "nc" not in _CACHE:
        _CACHE["nc"] = build_attention_nc()
    nc = _CACHE["nc"]

    in_maps = _make_in_maps(x, w_qkv, b_qkv, w_proj)
    # The device transiently wedges on some runs (INTERNAL error on a NEFF
    # that runs clean on retry). Retry, then rebuild once, before giving up.
    import os

    res = None
    for attempt in range(3):
        try:
            res = run_bass_kernel_spmd(nc, in_maps, core_ids=list(range(8)))
            break
        except Exception:
            if attempt == 2:
                raise
            os.environ.setdefault("NEURON_RT_RESET_CORES", "1")
            if attempt == 1:
                _CACHE.pop("nc", None)
                nc = _CACHE.setdefault("nc", build_attention_nc())

    outs = []
    for b in range(2):
        acc = res.results[b * 4]["out"].astype(np.float32).copy()
        for g in range(1, 4):
            acc += res.results[b * 4 + g]["out"]
        outs.append(acc)
    return (np.stack(outs) + b_proj.astype(np.float32)).astype(np.float32)

